# revision 38
# baseline (speedup 1.0000x reference)
"""Trainium2 Bass kernel for gaussian-weighted box-feature scatter (pooling).

Math (from the reference):
    out[c,h,w] = (1/N) * sum_n box_feats[c,n] * gmaps[n,h,w]
with gmaps separable:
    gmaps[n,h,w] = gy[n,h] * gx[n,w]

Host (tiny, O(N*C + N*(H+W) + N*H*C rank-factor prep)): box corner math, one
bilinear sample per box (box_feats [C,N]), the two 1-D gaussian profiles
gy [N,H], gx [N,W], and the premultiplied stationary factors
    B[n,h,c] = (box_feats[c,n]/N) * gy[n,h]   (fp16)
which ship to the device alongside gx (fp16) in one input DMA (~676 KB).

Device (heavy, O(C*H*W)): rank-N reconstruction
    out[c,h,:] = B[:,h,chalf].T @ gx
as 128 fp16 matmuls (stationary B slice via a ~105 ns standalone LDWEIGHTS
that the PE hides behind the previous matmul's streaming; moving gx), PSUM
f32 accumulate, fp16 PSUM->SBUF copy-casts, fp16 DMA writes. The f32->fp16
output is the big win: the kernel is write-bandwidth bound (per-core HBM
~358 GB/s), so halving output bytes halves the floor (16.8 MB/core ~ 47 us).
Host upcasts fp16 -> f32 while gathering.

Engine split (all under the ~47 us DMA window):
  PE:     128x (LDWEIGHTS + matmul [20,128]x[20,512] fp16)
  DVE:    even-h PSUM->SBUF double copies [128,1024] f32->fp16 (32x ~1.2 us)
  Scalar: odd-h double copies (32x ~1.1 us)
  SP:     1 input DMA + 7 chunked output DMAs (exactly 8 HWDGE lanes)

Each psum tile spans 2 banks: mm half0 -> cols 0:512, half1 -> 512:1024, so
one copy per h moves both c-halves. Output chunks [2,2,4,8,16,16,16] h-rows:
the first DMA issues ~2.5 us after the input lands; every chunk DMA covers
both halves via a strided dram AP. Per-chunk DVE "joiner" memsets plus a
post-assignment implied-wait elision keep every single-wait ISA struct
(Matmult, DMA descriptor) at one sync wait.

Sharding: H split across the 8 cores (64 rows each) - fully local.
"""

import numpy as np
from contextlib import ExitStack

from concourse import bass, tile, mybir
from concourse.tile import add_dep_helper
from concourse.bass_utils import run_bass_kernel_spmd

# Problem shapes (hardcoded per the task contract).
C, H, W = 256, 512, 512
N = 20
N_CORES = 8
HS = H // N_CORES          # 64 rows of the output per core
# Production is PE-paced at ~0.854 us/h-row; the DMA drains 0.734 us/h-row,
# so the stream finish time is max_k(ready_k + remaining drain). The taper
# below keeps every chunk's term under production_end + ~1.7 us: sizes obey
# s_k <= (10.2 - 0.12*c_{k-1}) / 0.734 with a small final chunk.
CHUNKS = [13, 11, 9, 8, 7, 6, 5, 3, 1, 1]   # h-rows per output DMA chunk
F32 = mybir.dt.float32
F16 = mybir.dt.float16
# Params live in three partition groups at bases 0/32/64 (the legal PE
# row-tile positions; quadrant 3 at 96 is unusable), each holding 22/22/20
# h-rows of premultiplied stationaries plus a copy of gx:
#   group g, partitions [32g, 32g+20): [B(nh x 256 c) | gx(512)] fp16.
# Three partition-sliced input DMAs load them concurrently.
HGRP = [(0, 0, 22), (32, 22, 22), (64, 44, 20)]   # (partition base, h0, nh)
B0 = W                     # B columns start after the gx block
PF = B0 + 22 * C           # [gx(512) | B(nh x 256 c)] per group

VOXEL = (0.4, 0.4, 4.0)
LIDAR_RANGE = (-102.4, -102.4, -3.0, 102.4, 102.4, 1.0)
DOWNSAMPLE = 1

_PROG = None          # cached Bass program
LAST_RESULTS = None   # BassKernelResults of the most recent run (for test.py)


def _host_factors(pred_box_infra, infra_features):
    """Per-box scalars, bilinear-sampled box features and separable gaussian
    profiles - all tiny. Coordinate math in float32 to match the reference
    bit-for-bit where it matters (floor/clip decisions)."""
    boxes = pred_box_infra[:N].astype(np.float32)
    feat = infra_features[0]                      # [C,H,W] float32
    l_corner = boxes.min(axis=1)                  # [N,3]
    r_corner = boxes.max(axis=1)
    sx = np.float32(VOXEL[0] * DOWNSAMPLE)
    sy = np.float32(VOXEL[1] * DOWNSAMPLE)
    x1 = (l_corner[:, 0] - np.float32(LIDAR_RANGE[0])) / sx
    y1 = (l_corner[:, 1] - np.float32(LIDAR_RANGE[1])) / sy
    x2 = (r_corner[:, 0] - np.float32(LIDAR_RANGE[0])) / sx
    y2 = (r_corner[:, 1] - np.float32(LIDAR_RANGE[1])) / sy
    bev_size = (y2 - y1) * (x2 - x1)              # [N]
    cx = np.float32(0.5) * (x1 + x2)
    cy = np.float32(0.5) * (y1 + y2)

    # bilinear sample at (cy, cx), matching the reference's clip/floor
    y = np.clip(cy, 0.0, H - 1.0).astype(np.float32)
    x = np.clip(cx, 0.0, W - 1.0).astype(np.float32)
    yl = np.floor(y).astype(np.int32)
    xl = np.floor(x).astype(np.int32)
    yh = np.minimum(yl + 1, H - 1)
    xh = np.minimum(xl + 1, W - 1)
    ly = (y - yl).astype(np.float64)[None, :]     # [1,N]
    lx = (x - xl).astype(np.float64)[None, :]
    g = lambda yi, xi: feat[:, yi, xi].astype(np.float64)   # [C,N]
    box_feats = (g(yl, xl) * (1 - ly) * (1 - lx)
                 + g(yl, xh) * (1 - ly) * lx
                 + g(yh, xl) * ly * (1 - lx)
                 + g(yh, xh) * ly * lx)           # [C,N] float64

    denom = 2.0 * bev_size.astype(np.float64) ** 2          # [N]
    hh = np.arange(H, dtype=np.float64)
    ww = np.arange(W, dtype=np.float64)
    gy = np.exp(-((hh[None, :] - x1.astype(np.float64)[:, None]) ** 2) / denom[:, None])
    gx = np.exp(-(ww[None, :] ** 2) / denom[:, None])

    a_t = np.ascontiguousarray((box_feats / N).T.astype(np.float32))  # [N,C]
    return a_t, gy.astype(np.float32), gx.astype(np.float32)


def _build_program(wm):
    """wm: truncated gaussian width — the device computes and writes only
    out[:, :, 0:wm]; the caller zero-fills the provably-negligible tail."""
    nc = bass.Bass("TRN2", target_bir_lowering=False, debug=False,
                   num_devices=N_CORES)
    params = nc.dram_tensor("params", [60, PF], F16, kind="ExternalInput").ap()
    out = nc.dram_tensor("out", [C, HS, wm], F16, kind="ExternalOutput").ap()
    # [c, b, h, w] view with c the 128-partition dim and b the c-half.
    out_v = out.rearrange("(b c) h w -> c b h w", b=2)

    with ExitStack() as ctx:
        tc = ctx.enter_context(tile.TileContext(nc))
        const = ctx.enter_context(tc.tile_pool(name="const", bufs=1))
        ppool = ctx.enter_context(tc.tile_pool(name="psum", bufs=4, space="PSUM"))
        # One stage pool per chunk size; bufs == #chunks of that size, so
        # stage slots are never recycled (no release waits needed at all).
        spools = {}
        for s in sorted(set(CHUNKS)):
            spools[s] = ctx.enter_context(
                tc.tile_pool(name=f"stage{s}", bufs=CHUNKS.count(s)))

        p_sb = const.tile([128, PF], F16)
        # Input DMAs, prioritized so compute starts ASAP: a small "head"
        # (first 6 h-rows of B for group 0, then gx) rides the DMA engines
        # alone and lands in ~0.3 us; the bulk follows while the PE already
        # streams. Sub-tile dep tracking scopes each LDWEIGHTS to the right
        # transfer.
        HEAD = B0 + 6 * C
        in_dmas = [
            nc.sync.dma_start(p_sb[0:N, 0:HEAD], params[0:N, 0:HEAD]),
            nc.sync.dma_start(p_sb[0:N, HEAD:PF], params[0:N, HEAD:PF]),
        ] + [
            nc.sync.dma_start(p_sb[base:base + N, :], params[20 * g:20 * g + N, :])
            for g, (base, _, _) in enumerate(HGRP) if base > 0
        ]
        # Scratch columns for the DVE joiner memsets (two per chunk).
        scratch = const.tile([128, 2 * len(CHUNKS)], F32)

        tail_deps = [dma.ins for dma in in_dmas]
        out_dmas = []
        N_IN = len(in_dmas)                    # HWDGE users before outputs
        h = 0
        for ci, s in enumerate(CHUNKS):
            # Stage layout per partition: [b(half)][h][w] so the DMA's SBUF
            # side merges (h,w) into one contiguous run and balances at 3D.
            stage = spools[s].tile([128, 2 * s * wm], F16, tag="stage")
            stage_v = stage[:].rearrange("p (b h w) -> p b h w", b=2, h=s)
            for l in range(s):
                base, h0, _ = next(gr for gr in HGRP
                                   if gr[1] <= h < gr[1] + gr[2])
                bcol = B0 + (h - h0) * C
                bg = p_sb[base:base + N, :]
                # PSUM halves keep the 512-f32 (one bank) stride so each
                # matmul's output stays inside a single bank.
                ps = ppool.tile([128, 2 * W], F32, tag="ps")
                nc.tensor.matmul(ps[:, 0:wm],
                                 bg[:, bcol:bcol + 128], bg[:, 0:wm],
                                 start=True, stop=True)
                mm = nc.tensor.matmul(ps[:, W:W + wm],
                                      bg[:, bcol + 128:bcol + C],
                                      bg[:, 0:wm],
                                      start=True, stop=True)
                ps_v = ps[:].rearrange("p (b w) -> p b w", b=2)[:, :, 0:wm]
                if h % 2 == 0:
                    cp = nc.vector.tensor_copy(stage_v[:, :, l, :], ps_v)
                else:
                    cp = nc.scalar.copy(stage_v[:, :, l, :], ps_v)
                    last_act_cp = cp
                h += 1
            # The chunk DMA depends on copies from both engines, but the
            # DMA descriptor holds ONE sync wait. Emit a DVE joiner that
            # waits on the chunk's last Act copy (DVE program order already
            # covers the DVE copies); the DMA then waits only the joiner's
            # DVE tick, and the implied Act wait is elided post-assignment.
            # When this chunk's HWDGE lane was used by an earlier OUTPUT
            # DMA, a second joiner observes that DMA's completion first so
            # the lane-reuse wait is likewise implied (input-lane reuse is
            # already implied through the LDWEIGHTS chain).
            prev_user = N_IN + ci - 8
            if prev_user >= N_IN:
                j2 = nc.vector.memset(
                    scratch[:, len(CHUNKS) + ci:len(CHUNKS) + ci + 1], 0.0)
                add_dep_helper(j2.ins, out_dmas[prev_user - N_IN].ins,
                               sync=True, reason="lane reuse joiner")
            joiner = nc.vector.memset(scratch[:, ci:ci + 1], 0.0)
            add_dep_helper(joiner.ins, last_act_cp.ins, sync=True,
                           reason="chunk copy joiner")
            dma = nc.sync.dma_start(out_v[:, :, h - s:h, :], stage_v)
            add_dep_helper(dma.ins, joiner.ins, sync=True,
                           reason="dma waits joiner")
            out_dmas.append(dma)
            tail_deps.append(dma.ins)

        # Tail drain pre-cover: one single-wait SP nop per outstanding sem
        # so the drain itself needs no multi-wait instruction.
        tail_deps += [mm.ins, cp.ins, joiner.ins]
        for dep in tail_deps:
            tnop = nc.sync.nop(nofuse=True)
            add_dep_helper(tnop.ins, dep, sync=True,
                           reason="tail drain pre-cover")
    _elide_implied_waits(nc, tc)
    return nc


def _elide_implied_waits(nc, tc):
    """Several ISA structs (Matmult, TensorScalar, DMA_DIRECT2D) hold ONE
    sync wait, but Tile sometimes assigns two:
      - PSUM slot recycling puts both the PSUM->SBUF copy's tick and a WAW
        "previous writer retired" PE self-wait on the reusing matmul, yet
        the copy itself already waits for that PE tick;
      - a chunk DMA waits on both copy engines, yet its DVE joiner already
        waits the Act tick.
    A wait (S >= v) is provably redundant when reaching another wait's value
    transitively guarantees it: completion of that wait's producer implies
    its own waits held, and every earlier producer on the same semaphore
    retired, recursively. Compute that closure and drop only implied waits.
    (The transitivity matters for HWDGE lane reuse: an output DMA reusing an
    input DMA's lane inherits a second wait on the input's completion, which
    any copy tick implies via copy -> matmul -> ldweights -> input.)"""
    # Walk the scheduled stream once. For each instruction, fold its waits
    # (and their closures) into its ENGINE's running implication dict —
    # engine program order means a later instruction's completion implies
    # every earlier same-engine instruction's waits held. Each semaphore
    # update then snapshots engine-dict + same-sem history as the closure
    # of (sem, value).
    closure = {}          # sem -> list of (value, {sem': implied_value})
    sem_acc = {}          # sem -> accumulated implication dict
    eng_acc = {}          # engine -> accumulated implication dict
    cum = {}

    def closure_of(sem, val):
        best = None
        for v2, im in closure.get(sem, []):
            if v2 <= val:
                best = im
        return best or {}

    def merge(dst, src):
        for s, v in src.items():
            if dst.get(s, 0) < v:
                dst[s] = v

    for insts in tc.ordered_instructions_by_block.values():
        for inst in insts:
            si = inst.sync_info
            if si is None:
                continue
            acc = eng_acc.setdefault(str(inst.engine), {})
            for w in si.on_wait:
                if acc.get(w.ant_name, 0) < w.wait_value:
                    acc[w.ant_name] = w.wait_value
                merge(acc, closure_of(w.ant_name, w.wait_value))
            for u in si.on_update:
                sem = u.ant_name
                cum[sem] = cum.get(sem, 0) + (u.update_value or 1)
                snap = dict(acc)
                merge(snap, sem_acc.get(sem, {}))
                snap[sem] = max(snap.get(sem, 0), cum[sem])
                sem_acc[sem] = snap
                closure.setdefault(sem, []).append((cum[sem], snap))

    def implied(keep, w):
        """True if wait `w` is implied by `keep` (S >= v) being reached."""
        return closure_of(keep.ant_name, keep.wait_value).get(w.ant_name, 0) \
            >= w.wait_value

    for inst in nc.inst_map.values():
        si = inst.sync_info
        if si is None or len(si.on_wait) < 2:
            continue
        waits = list(si.on_wait)
        changed = True
        while changed and len(waits) > 1:
            changed = False
            for w in waits:
                if any(k is not w and implied(k, w) for k in waits):
                    waits.remove(w)
                    changed = True
                    break
        if len(waits) != len(si.on_wait):
            si.on_wait = waits
            inst.sync_info = si


def _program(wm):
    global _PROG
    if _PROG is None:
        _PROG = {}
    if wm not in _PROG:
        _PROG[wm] = _build_program(wm)
    return _PROG[wm]


def _w_cut(b_full, gx):
    """Smallest multiple of 32 such that the truncated gaussian tail is
    provably negligible: bound the absolute truncation error at any (c,h,w)
    by sum_n max|B_n| * gx[n,w], and require it under 30% of the tolerance
    (2e-2) times a computable lower bound on the output scale (the w=0
    column of the output)."""
    coef = np.abs(b_full.astype(np.float32)).max(axis=(1, 2))      # [N]
    tail = (coef[:, None] * gx).sum(axis=0)                        # [W]
    scale_lb = np.abs(b_full.astype(np.float32).sum(axis=0)).max()
    thresh = 0.30 * 2e-2 * scale_lb
    ok = np.where(tail <= thresh)[0]
    wm = int(ok[0]) if len(ok) else W
    while len(ok) and wm < W and tail[wm:].max() > thresh:
        wm += 1
    wm = min(W, max(64, -(-wm // 32) * 32))
    return wm


def make_in_maps(pred_box_infra, infra_features):
    a_t, gy_full, gx = _host_factors(
        np.asarray(pred_box_infra, dtype=np.float32),
        np.asarray(infra_features, dtype=np.float32),
    )
    # B[n, h, c] = a_t[n, c] * gy[n, h], shipped premultiplied in fp16 in
    # three h-groups (one per PE row-tile position), each with a gx copy.
    b_full = (gy_full[:, :, None] * a_t[:, None, :]).astype(np.float16)
    gx16 = gx.astype(np.float16)
    maps = []
    for c in range(N_CORES):
        p = np.zeros((60, PF), dtype=np.float16)
        b_core = b_full[:, c * HS:(c + 1) * HS, :]       # [N, HS, C]
        for g, (_, h0, nh) in enumerate(HGRP):
            p[20 * g:20 * g + N, 0:W] = gx16
            p[20 * g:20 * g + N, B0:B0 + nh * C] = \
                b_core[:, h0:h0 + nh, :].reshape(N, nh * C)
        maps.append({"params": p})
    return maps, _w_cut(b_full, gx)


def kernel(pred_box_infra, infra_features):
    global LAST_RESULTS
    in_maps, wm = make_in_maps(pred_box_infra, infra_features)
    nc = _program(wm)
    res = run_bass_kernel_spmd(nc, in_maps, core_ids=list(range(N_CORES)))
    LAST_RESULTS = res
    full = np.zeros((1, C, H, W), dtype=np.float32)
    for c in range(N_CORES):
        full[0, :, c * HS:(c + 1) * HS, 0:wm] = \
            res.results[c]["out"].astype(np.float32)
    return full


# revision 40
# speedup vs baseline: 1.2240x; 1.2240x over previous
"""Trainium2 Bass kernel for gaussian-weighted box-feature scatter (pooling).

Math (from the reference):
    out[c,h,w] = (1/N) * sum_n box_feats[c,n] * gmaps[n,h,w]
with gmaps separable:
    gmaps[n,h,w] = gy[n,h] * gx[n,w]

Host (tiny, O(N*C + N*(H+W) + N*H*C rank-factor prep)): box corner math, one
bilinear sample per box (box_feats [C,N]), the two 1-D gaussian profiles
gy [N,H], gx [N,W], and the premultiplied stationary factors
    B[n,h,c] = (box_feats[c,n]/N) * gy[n,h]   (fp16)
which ship to the device alongside gx (fp16) in one input DMA (~676 KB).

Device (heavy, O(C*H*W)): rank-N reconstruction
    out[c,h,:] = B[:,h,chalf].T @ gx
as 128 fp16 matmuls (stationary B slice via a ~105 ns standalone LDWEIGHTS
that the PE hides behind the previous matmul's streaming; moving gx), PSUM
f32 accumulate, fp16 PSUM->SBUF copy-casts, fp16 DMA writes. The f32->fp16
output is the big win: the kernel is write-bandwidth bound (per-core HBM
~358 GB/s), so halving output bytes halves the floor (16.8 MB/core ~ 47 us).
Host upcasts fp16 -> f32 while gathering.

Engine split (all under the ~47 us DMA window):
  PE:     128x (LDWEIGHTS + matmul [20,128]x[20,512] fp16)
  DVE:    even-h PSUM->SBUF double copies [128,1024] f32->fp16 (32x ~1.2 us)
  Scalar: odd-h double copies (32x ~1.1 us)
  SP:     1 input DMA + 7 chunked output DMAs (exactly 8 HWDGE lanes)

Each psum tile spans 2 banks: mm half0 -> cols 0:512, half1 -> 512:1024, so
one copy per h moves both c-halves. Output chunks [2,2,4,8,16,16,16] h-rows:
the first DMA issues ~2.5 us after the input lands; every chunk DMA covers
both halves via a strided dram AP. Per-chunk DVE "joiner" memsets plus a
post-assignment implied-wait elision keep every single-wait ISA struct
(Matmult, DMA descriptor) at one sync wait.

Sharding: H split across the 8 cores (64 rows each) - fully local.
"""

import numpy as np
from contextlib import ExitStack

from concourse import bass, tile, mybir
from concourse.tile import add_dep_helper
from concourse.bass_utils import run_bass_kernel_spmd

# Problem shapes (hardcoded per the task contract).
C, H, W = 256, 512, 512
N = 20
N_CORES = 8
HS = H // N_CORES          # 64 rows of the output per core
# Production is PE-paced at ~0.854 us/h-row; the DMA drains 0.734 us/h-row,
# so the stream finish time is max_k(ready_k + remaining drain). The taper
# below keeps every chunk's term under production_end + ~1.7 us: sizes obey
# s_k <= (10.2 - 0.12*c_{k-1}) / 0.734 with a small final chunk.
CHUNKS = [13, 11, 9, 8, 7, 6, 5, 3, 1, 1]   # h-rows per output DMA chunk
F32 = mybir.dt.float32
F16 = mybir.dt.float16
# Params live in three partition groups at bases 0/32/64 (the legal PE
# row-tile positions; quadrant 3 at 96 is unusable), each holding 22/22/20
# h-rows of premultiplied stationaries plus a copy of gx:
#   group g, partitions [32g, 32g+20): [B(nh x 256 c) | gx(512)] fp16.
# Three partition-sliced input DMAs load them concurrently.
HGRP = [(0, 0, 22), (32, 22, 22), (64, 44, 20)]   # (partition base, h0, nh)
B0 = W                     # B columns start after the gx block
PF = B0 + 22 * C           # [gx(512) | B(nh x 256 c)] per group

VOXEL = (0.4, 0.4, 4.0)
LIDAR_RANGE = (-102.4, -102.4, -3.0, 102.4, 102.4, 1.0)
DOWNSAMPLE = 1

_PROG = None          # cached Bass program
LAST_RESULTS = None   # BassKernelResults of the most recent run (for test.py)


def _host_factors(pred_box_infra, infra_features):
    """Per-box scalars, bilinear-sampled box features and separable gaussian
    profiles - all tiny. Coordinate math in float32 to match the reference
    bit-for-bit where it matters (floor/clip decisions)."""
    boxes = pred_box_infra[:N].astype(np.float32)
    feat = infra_features[0]                      # [C,H,W] float32
    l_corner = boxes.min(axis=1)                  # [N,3]
    r_corner = boxes.max(axis=1)
    sx = np.float32(VOXEL[0] * DOWNSAMPLE)
    sy = np.float32(VOXEL[1] * DOWNSAMPLE)
    x1 = (l_corner[:, 0] - np.float32(LIDAR_RANGE[0])) / sx
    y1 = (l_corner[:, 1] - np.float32(LIDAR_RANGE[1])) / sy
    x2 = (r_corner[:, 0] - np.float32(LIDAR_RANGE[0])) / sx
    y2 = (r_corner[:, 1] - np.float32(LIDAR_RANGE[1])) / sy
    bev_size = (y2 - y1) * (x2 - x1)              # [N]
    cx = np.float32(0.5) * (x1 + x2)
    cy = np.float32(0.5) * (y1 + y2)

    # bilinear sample at (cy, cx), matching the reference's clip/floor
    y = np.clip(cy, 0.0, H - 1.0).astype(np.float32)
    x = np.clip(cx, 0.0, W - 1.0).astype(np.float32)
    yl = np.floor(y).astype(np.int32)
    xl = np.floor(x).astype(np.int32)
    yh = np.minimum(yl + 1, H - 1)
    xh = np.minimum(xl + 1, W - 1)
    ly = (y - yl).astype(np.float64)[None, :]     # [1,N]
    lx = (x - xl).astype(np.float64)[None, :]
    g = lambda yi, xi: feat[:, yi, xi].astype(np.float64)   # [C,N]
    box_feats = (g(yl, xl) * (1 - ly) * (1 - lx)
                 + g(yl, xh) * (1 - ly) * lx
                 + g(yh, xl) * ly * (1 - lx)
                 + g(yh, xh) * ly * lx)           # [C,N] float64

    denom = 2.0 * bev_size.astype(np.float64) ** 2          # [N]
    hh = np.arange(H, dtype=np.float64)
    ww = np.arange(W, dtype=np.float64)
    gy = np.exp(-((hh[None, :] - x1.astype(np.float64)[:, None]) ** 2) / denom[:, None])
    gx = np.exp(-(ww[None, :] ** 2) / denom[:, None])

    a_t = np.ascontiguousarray((box_feats / N).T.astype(np.float32))  # [N,C]
    return a_t, gy.astype(np.float32), gx.astype(np.float32)


def _build_program(wm):
    """wm: truncated gaussian width — the device computes and writes only
    out[:, :, 0:wm]; the caller zero-fills the provably-negligible tail."""
    nc = bass.Bass("TRN2", target_bir_lowering=False, debug=False,
                   num_devices=N_CORES)
    params = nc.dram_tensor("params", [60, PF], F16, kind="ExternalInput").ap()
    out = nc.dram_tensor("out", [C, HS, wm], F16, kind="ExternalOutput").ap()
    # [c, b, h, w] view with c the 128-partition dim and b the c-half.
    out_v = out.rearrange("(b c) h w -> c b h w", b=2)

    with ExitStack() as ctx:
        tc = ctx.enter_context(tile.TileContext(nc))
        const = ctx.enter_context(tc.tile_pool(name="const", bufs=1))
        ppool = ctx.enter_context(tc.tile_pool(name="psum", bufs=4, space="PSUM"))
        # One stage pool per chunk size; bufs == #chunks of that size, so
        # stage slots are never recycled (no release waits needed at all).
        spools = {}
        for s in sorted(set(CHUNKS)):
            spools[s] = ctx.enter_context(
                tc.tile_pool(name=f"stage{s}", bufs=CHUNKS.count(s)))

        p_sb = const.tile([128, PF], F16)
        # Input DMAs, prioritized so compute starts ASAP: a small "head"
        # (first 6 h-rows of B for group 0, then gx) rides the DMA engines
        # alone and lands in ~0.3 us; the bulk follows while the PE already
        # streams. Sub-tile dep tracking scopes each LDWEIGHTS to the right
        # transfer.
        HEAD = B0 + 2 * C
        in_dmas = [
            nc.sync.dma_start(p_sb[0:N, 0:HEAD], params[0:N, 0:HEAD]),
            nc.sync.dma_start(p_sb[0:N, HEAD:PF], params[0:N, HEAD:PF]),
        ] + [
            nc.sync.dma_start(p_sb[base:base + N, :], params[20 * g:20 * g + N, :])
            for g, (base, _, _) in enumerate(HGRP) if base > 0
        ]
        # Scratch columns for the DVE joiner memsets (two per chunk).
        scratch = const.tile([128, 2 * len(CHUNKS)], F32)

        tail_deps = [dma.ins for dma in in_dmas]
        out_dmas = []
        N_IN = len(in_dmas)                    # HWDGE users before outputs
        h = 0
        for ci, s in enumerate(CHUNKS):
            # Stage layout per partition: [b(half)][h][w] so the DMA's SBUF
            # side merges (h,w) into one contiguous run and balances at 3D.
            stage = spools[s].tile([128, 2 * s * wm], F16, tag="stage")
            stage_v = stage[:].rearrange("p (b h w) -> p b h w", b=2, h=s)
            for l in range(s):
                base, h0, _ = next(gr for gr in HGRP
                                   if gr[1] <= h < gr[1] + gr[2])
                bcol = B0 + (h - h0) * C
                bg = p_sb[base:base + N, :]
                # PSUM halves keep the 512-f32 (one bank) stride so each
                # matmul's output stays inside a single bank.
                ps = ppool.tile([128, 2 * W], F32, tag="ps")
                nc.tensor.matmul(ps[:, 0:wm],
                                 bg[:, bcol:bcol + 128], bg[:, 0:wm],
                                 start=True, stop=True)
                mm = nc.tensor.matmul(ps[:, W:W + wm],
                                      bg[:, bcol + 128:bcol + C],
                                      bg[:, 0:wm],
                                      start=True, stop=True)
                ps_v = ps[:].rearrange("p (b w) -> p b w", b=2)[:, :, 0:wm]
                if h % 2 == 0:
                    cp = nc.vector.tensor_copy(stage_v[:, :, l, :], ps_v)
                else:
                    cp = nc.scalar.copy(stage_v[:, :, l, :], ps_v)
                    last_act_cp = cp
                h += 1
            # The chunk DMA depends on copies from both engines, but the
            # DMA descriptor holds ONE sync wait. Emit a DVE joiner that
            # waits on the chunk's last Act copy (DVE program order already
            # covers the DVE copies); the DMA then waits only the joiner's
            # DVE tick, and the implied Act wait is elided post-assignment.
            # When this chunk's HWDGE lane was used by an earlier OUTPUT
            # DMA, a second joiner observes that DMA's completion first so
            # the lane-reuse wait is likewise implied (input-lane reuse is
            # already implied through the LDWEIGHTS chain).
            prev_user = N_IN + ci - 8
            if prev_user >= N_IN:
                j2 = nc.vector.memset(
                    scratch[:, len(CHUNKS) + ci:len(CHUNKS) + ci + 1], 0.0)
                add_dep_helper(j2.ins, out_dmas[prev_user - N_IN].ins,
                               sync=True, reason="lane reuse joiner")
            joiner = nc.vector.memset(scratch[:, ci:ci + 1], 0.0)
            add_dep_helper(joiner.ins, last_act_cp.ins, sync=True,
                           reason="chunk copy joiner")
            dma = nc.sync.dma_start(out_v[:, :, h - s:h, :], stage_v)
            add_dep_helper(dma.ins, joiner.ins, sync=True,
                           reason="dma waits joiner")
            out_dmas.append(dma)
            tail_deps.append(dma.ins)

        # Tail drain pre-cover: one single-wait SP nop per outstanding sem
        # so the drain itself needs no multi-wait instruction.
        tail_deps += [mm.ins, cp.ins, joiner.ins]
        for dep in tail_deps:
            tnop = nc.sync.nop(nofuse=True)
            add_dep_helper(tnop.ins, dep, sync=True,
                           reason="tail drain pre-cover")
    _elide_implied_waits(nc, tc)
    return nc


def _elide_implied_waits(nc, tc):
    """Several ISA structs (Matmult, TensorScalar, DMA_DIRECT2D) hold ONE
    sync wait, but Tile sometimes assigns two:
      - PSUM slot recycling puts both the PSUM->SBUF copy's tick and a WAW
        "previous writer retired" PE self-wait on the reusing matmul, yet
        the copy itself already waits for that PE tick;
      - a chunk DMA waits on both copy engines, yet its DVE joiner already
        waits the Act tick.
    A wait (S >= v) is provably redundant when reaching another wait's value
    transitively guarantees it: completion of that wait's producer implies
    its own waits held, and every earlier producer on the same semaphore
    retired, recursively. Compute that closure and drop only implied waits.
    (The transitivity matters for HWDGE lane reuse: an output DMA reusing an
    input DMA's lane inherits a second wait on the input's completion, which
    any copy tick implies via copy -> matmul -> ldweights -> input.)"""
    # Walk the scheduled stream once. For each instruction, fold its waits
    # (and their closures) into its ENGINE's running implication dict —
    # engine program order means a later instruction's completion implies
    # every earlier same-engine instruction's waits held. Each semaphore
    # update then snapshots engine-dict + same-sem history as the closure
    # of (sem, value).
    closure = {}          # sem -> list of (value, {sem': implied_value})
    sem_acc = {}          # sem -> accumulated implication dict
    eng_acc = {}          # engine -> accumulated implication dict
    cum = {}

    def closure_of(sem, val):
        best = None
        for v2, im in closure.get(sem, []):
            if v2 <= val:
                best = im
        return best or {}

    def merge(dst, src):
        for s, v in src.items():
            if dst.get(s, 0) < v:
                dst[s] = v

    for insts in tc.ordered_instructions_by_block.values():
        for inst in insts:
            si = inst.sync_info
            if si is None:
                continue
            acc = eng_acc.setdefault(str(inst.engine), {})
            for w in si.on_wait:
                if acc.get(w.ant_name, 0) < w.wait_value:
                    acc[w.ant_name] = w.wait_value
                merge(acc, closure_of(w.ant_name, w.wait_value))
            for u in si.on_update:
                sem = u.ant_name
                cum[sem] = cum.get(sem, 0) + (u.update_value or 1)
                snap = dict(acc)
                merge(snap, sem_acc.get(sem, {}))
                snap[sem] = max(snap.get(sem, 0), cum[sem])
                sem_acc[sem] = snap
                closure.setdefault(sem, []).append((cum[sem], snap))

    def implied(keep, w):
        """True if wait `w` is implied by `keep` (S >= v) being reached."""
        return closure_of(keep.ant_name, keep.wait_value).get(w.ant_name, 0) \
            >= w.wait_value

    for inst in nc.inst_map.values():
        si = inst.sync_info
        if si is None or len(si.on_wait) < 2:
            continue
        waits = list(si.on_wait)
        changed = True
        while changed and len(waits) > 1:
            changed = False
            for w in waits:
                if any(k is not w and implied(k, w) for k in waits):
                    waits.remove(w)
                    changed = True
                    break
        if len(waits) != len(si.on_wait):
            si.on_wait = waits
            inst.sync_info = si


def _program(wm):
    global _PROG
    if _PROG is None:
        _PROG = {}
    if wm not in _PROG:
        _PROG[wm] = _build_program(wm)
    return _PROG[wm]


def _w_cut(b_full, gx):
    """Smallest multiple of 32 such that the truncated gaussian tail is
    provably negligible: bound the absolute truncation error at any (c,h,w)
    by sum_n max_c|B[n,h,c]| * gx[n,w] (exact per-h worst case), and require
    it under 50% of the tolerance (2e-2) times a computable lower bound on
    the output scale (the w=0 column of the output)."""
    bf = b_full.astype(np.float32)
    prof = np.abs(bf).max(axis=2)                                  # [N,H]
    colmax = np.einsum('nh,nw->hw', prof, gx).max(axis=0)          # [W]
    scale_lb = np.abs(bf.sum(axis=0)).max()
    thresh = 0.50 * 2e-2 * scale_lb
    # running max of the tail from each w onward
    tailmax = np.maximum.accumulate(colmax[::-1])[::-1]
    ok = np.where(tailmax <= thresh)[0]
    wm = int(ok[0]) if len(ok) else W
    wm = min(W, max(64, -(-wm // 32) * 32))
    return wm


def make_in_maps(pred_box_infra, infra_features):
    a_t, gy_full, gx = _host_factors(
        np.asarray(pred_box_infra, dtype=np.float32),
        np.asarray(infra_features, dtype=np.float32),
    )
    # B[n, h, c] = a_t[n, c] * gy[n, h], shipped premultiplied in fp16 in
    # three h-groups (one per PE row-tile position), each with a gx copy.
    b_full = (gy_full[:, :, None] * a_t[:, None, :]).astype(np.float16)
    gx16 = gx.astype(np.float16)
    maps = []
    for c in range(N_CORES):
        p = np.zeros((60, PF), dtype=np.float16)
        b_core = b_full[:, c * HS:(c + 1) * HS, :]       # [N, HS, C]
        for g, (_, h0, nh) in enumerate(HGRP):
            p[20 * g:20 * g + N, 0:W] = gx16
            p[20 * g:20 * g + N, B0:B0 + nh * C] = \
                b_core[:, h0:h0 + nh, :].reshape(N, nh * C)
        maps.append({"params": p})
    return maps, _w_cut(b_full, gx)


def kernel(pred_box_infra, infra_features):
    global LAST_RESULTS
    in_maps, wm = make_in_maps(pred_box_infra, infra_features)
    nc = _program(wm)
    res = run_bass_kernel_spmd(nc, in_maps, core_ids=list(range(N_CORES)))
    LAST_RESULTS = res
    full = np.zeros((1, C, H, W), dtype=np.float32)
    for c in range(N_CORES):
        full[0, :, c * HS:(c + 1) * HS, 0:wm] = \
            res.results[c]["out"].astype(np.float32)
    return full
